# revision 26
# baseline (speedup 1.0000x reference)
"""Single-head causal attention (B=8, T=4096, EMB=1024, HEAD=64) on 8 trn2 cores.

Strategy: data-parallel over batch, one batch element per NeuronCore.

v2: t-tile-streamed pipeline (8 tiles of 512), designed so the ACT engine's
exp stream starts ~5us in and every engine stays continuously busy (HAM warm):

Per t-tile j (all matmuls bf16, fp32 PSUM):
  1. DMA x tile j (one contiguous 1MB transfer, host layout [j][p][k][c]).
  2. KQ^T tile: 8 matmuls, w_kq chunk stationary, xt moving 512-wide.
     K^T -> ks_sb low partitions, Q^T -> qs_sb high partitions (DVE), then
     sbuf-sbuf DMA duplicates each onto the other partition half so score
     matmuls can be packed two-per-PE-pass via tile_position row tiling
     (contraction is only d=64, so two independent 64-row matmuls share
     the 128x128 array).
  3. V^T tile via w_v stationary (64-wide) with 2x col tiling (two 256-col
     half-tiles concurrently), then 4 PE transposes -> V natural chunks
     [128s, 64d] stored next to a preset ones column (for the softmax
     denominator).
  4. Scores S^T chunk (a) = K chunk @ Q^T tile, row-tiled in (lo,hi) pairs.
     exp on ACT in 3-chunk [128,1536] groups straight out of PSUM (scale
     1/8 folded in), bf16 out to pt. Sub-diagonal garbage zeroed (GpSimd)
     and the diagonal 128x128 block masked (DVE).
  5. PV transposed: out^T[65, 512] += [V_aa | ones]^T @ P^T[aa] per s-chunk
     aa <= 4j+3; V stationary is only 65 columns so LDWEIGHTS hides under
     the 512-wide moving pass (the v1 kernel lost ~60us to per-matmul
     128-column weight loads here). Runs one exp-group behind scores.
  6. out^T tile -> SBUF fp32 -> DRAM [65, T]. Host divides by the Z row
     and transposes (no device normalization on the critical path).
"""

import os

import numpy as np
import ml_dtypes

B, T, EMB, HEAD = 8, 4096, 1024, 64
KCH = EMB // 128           # 8 contraction chunks
NTT = T // 512             # 8 t-tiles
NTS = T // 128             # 32 s-chunks
BF16 = ml_dtypes.bfloat16

# feature flags (bisect aids; defaults = full-speed configuration)
# NOTE: col tiling (tile_position=(0, 64)) crashes trn2 hw — never use it.
SC_PAIR = os.environ.get("BASS_SC_PAIR", "1") != "0"    # row-tiled score pairs
VNAT = os.environ.get("BASS_VNAT", "transpose")          # "transpose" | "direct"
# NOTE: is_transpose at base partition 64 also crashes hw; keep transposes
# on partitions 0:64 (TR_PAIR=0).
TR_PAIR = os.environ.get("BASS_TR_PAIR", "0") != "0"

_CACHE = {}


def _build_program():
    import concourse.bacc as bacc
    import concourse.tile as tile
    from concourse import mybir
    from concourse.masks import make_identity

    fp32 = mybir.dt.float32
    bf16 = mybir.dt.bfloat16
    EXP = mybir.ActivationFunctionType.Exp

    nc = bacc.Bacc("TRN2", target_bir_lowering=False, debug=False)
    xj_ap = nc.dram_tensor("xj", [NTT, 128, KCH, 512], bf16, kind="ExternalInput").ap()
    # per k-chunk 256 cols: [Wk | Wq] then [Wv | Wk] — the second pass puts
    # V^T on partitions 0:64 and a second copy of K^T on 64:128, which the
    # row-tiled score pairs need (saves a partition-shift DMA per tile)
    w_ap = nc.dram_tensor("w", [128, KCH * 256], bf16, kind="ExternalInput").ap()
    mask_ap = nc.dram_tensor("mask", [128, 128], bf16, kind="ExternalInput").ap()
    o_ap = nc.dram_tensor("o", [65, T], fp32, kind="ExternalOutput").ap()

    with tile.TileContext(nc) as tc:
        with (
            tc.tile_pool(name="consts", bufs=1) as consts,
            tc.tile_pool(name="ps_scr", bufs=1, space="PSUM") as ps_scr,
            tc.tile_pool(name="ps_sc", bufs=2, space="PSUM") as ps_sc,
            tc.tile_pool(name="ps_pv", bufs=1, space="PSUM") as ps_pv,
        ):
            # ---------- constants ----------
            w_sb = consts.tile([128, KCH * 256], bf16, tag="w")
            nc.sync.dma_start(out=w_sb, in_=w_ap)
            mask_sb = consts.tile([128, 128], bf16, tag="mask")
            nc.sync.dma_start(out=mask_sb, in_=mask_ap)

            xt_sb = consts.tile([128, NTT, KCH, 512], bf16, tag="xt")
            # tile 0 split per k-chunk so KQ_0 starts on the first 128KB
            for k in range(KCH):
                nc.sync.dma_start(out=xt_sb[:, 0, k], in_=xj_ap[0, :, k])
            for j in range(1, NTT):
                nc.sync.dma_start(out=xt_sb[:, j], in_=xj_ap[j])

            # identity (fp32) for PE transposes (partitions 0:64 only;
            # is_transpose at base partition 64 crashes hw)
            ident_sb = consts.tile([128, 64], fp32, tag="ident")
            make_identity(nc, ident_sb[0:64, :])
            if TR_PAIR:
                nc.sync.dma_start(out=ident_sb[64:128, :], in_=ident_sb[0:64, :])

            # V natural chunks [128s, 64d | ones] per s-chunk
            vt_sb = consts.tile([128, NTS * 65], bf16, tag="vt")
            nc.gpsimd.memset(vt_sb, 1.0)

            ks_sb = consts.tile([128, T], bf16, tag="ks")   # K^T on both halves
            qs_sb = consts.tile([128, T], bf16, tag="qs")   # Q^T on both halves
            vts_sb = consts.tile([64, NTT * 512], fp32, tag="vts")  # V^T staging
            pt_sb = consts.tile([128, NTS * 512], bf16, tag="pt")    # P^T chunks
            ot_sb = consts.tile([65, T], fp32, tag="ot")             # out^T

            for j in range(NTT):
                jsl = slice(j * 512, (j + 1) * 512)

                # ---- KQ^T tile ----
                scr = ps_scr.tile([128, 512], fp32, tag="scr")
                for k in range(KCH):
                    nc.tensor.matmul(
                        scr,
                        w_sb[:, k * 256:k * 256 + 128],
                        xt_sb[:, j, k],
                        start=(k == 0),
                        stop=(k == KCH - 1),
                        skip_group_check=True,
                    )
                nc.vector.tensor_copy(ks_sb[0:64, jsl], scr[0:64, :])
                nc.vector.tensor_copy(qs_sb[64:128, jsl], scr[64:128, :])
                # Q^T -> low partitions via gpsimd software DGE (own DMA
                # queue; the SP queue is busy streaming x tiles)
                nc.gpsimd.dma_start(out=qs_sb[0:64, jsl], in_=qs_sb[64:128, jsl])

                if VNAT == "direct":
                    # baseline-style: xt chunk stationary, w_v moving
                    if SC_PAIR:
                        nc.gpsimd.dma_start(
                            out=ks_sb[64:128, jsl], in_=ks_sb[0:64, jsl]
                        )
                    for k in range(KCH):
                        for q in range(4):
                            nc.tensor.matmul(
                                scr[:, 256 + q * 64:256 + q * 64 + 64],
                                xt_sb[:, j, k, q * 128:(q + 1) * 128],
                                w_sb[:, k * 256 + 128:k * 256 + 192],
                                start=(k == 0 and q == 0),
                                stop=(k == KCH - 1),
                                skip_group_check=True,
                            )
                    for q in range(4):
                        i = 4 * j + q
                        nc.vector.tensor_copy(
                            vt_sb[:, i * 65:i * 65 + 64],
                            scr[:, 256 + q * 64:256 + q * 64 + 64],
                        )
                else:
                    # ---- [V^T ; K^T] tile ----
                    for k in range(KCH):
                        nc.tensor.matmul(
                            scr,
                            w_sb[:, k * 256 + 128:k * 256 + 256],
                            xt_sb[:, j, k],
                            start=(k == 0),
                            stop=(k == KCH - 1),
                            skip_group_check=True,
                        )
                    nc.vector.tensor_copy(
                        vts_sb[:, j * 512:(j + 1) * 512], scr[0:64, :]
                    )
                    nc.vector.tensor_copy(ks_sb[64:128, jsl], scr[64:128, :])

                    # ---- V natural via PE transposes ----
                    for q in range(4):
                        half = 0
                        src = vts_sb[half:half + 64,
                                     j * 512 + q * 128:j * 512 + q * 128 + 128]
                        nc.tensor.matmul(
                            scr[:, 256 + q * 64:256 + q * 64 + 64],
                            src,
                            ident_sb[half:half + 64, :],
                            is_transpose=True,
                            start=(q == 0),
                            stop=(q == 3),
                            skip_group_check=True,
                        )
                    for q in range(4):
                        i = 4 * j + q
                        nc.vector.tensor_copy(
                            vt_sb[:, i * 65:i * 65 + 64],
                            scr[:, 256 + q * 64:256 + q * 64 + 64],
                        )

                # ---- scores + exp + PV, pipelined by one exp-group ----
                po = ps_pv.tile([65, 512], fp32, tag="pv")
                nchunk = 4 * j + 4
                groups = [list(range(g, min(g + 3, nchunk))) for g in range(0, nchunk, 3)]

                def emit_pv(chunks, j=j, po=po, nchunk=nchunk):
                    for aa in chunks:
                        nc.tensor.matmul(
                            po,
                            vt_sb[:, aa * 65:(aa + 1) * 65],
                            pt_sb[:, aa * 512:(aa + 1) * 512],
                            start=(aa == 0),
                            stop=(aa == nchunk - 1),
                            skip_group_check=True,
                        )

                for gi, chunks in enumerate(groups):
                    sc = ps_sc.tile([128, 1536], fp32, tag="sc")
                    for ci, a in enumerate(chunks):
                        half = 64 if (SC_PAIR and a % 2 == 1) else 0
                        nc.tensor.matmul(
                            sc[:, ci * 512:(ci + 1) * 512],
                            ks_sb[half:half + 64, a * 128:(a + 1) * 128],
                            qs_sb[half:half + 64, jsl],
                            start=True,
                            stop=True,
                            skip_group_check=True,
                        )
                    cnt = len(chunks)
                    a0 = chunks[0]
                    nc.scalar.activation(
                        pt_sb[:, a0 * 512:(a0 + cnt) * 512],
                        sc[:, 0:cnt * 512],
                        EXP,
                        scale=0.125,
                    )
                    for a in chunks:
                        if a >= 4 * j:
                            sub = a - 4 * j
                            if sub > 0:
                                nc.gpsimd.memset(
                                    pt_sb[:, a * 512:a * 512 + 128 * sub], 0.0
                                )
                            dsl = slice(a * 512 + 128 * sub, a * 512 + 128 * sub + 128)
                            nc.vector.tensor_mul(pt_sb[:, dsl], pt_sb[:, dsl], mask_sb)
                    if gi >= 1:
                        emit_pv(groups[gi - 1])
                emit_pv(groups[-1])

                nc.vector.tensor_copy(ot_sb[:, jsl], po)
                nc.sync.dma_start(out=o_ap[:, jsl], in_=ot_sb[:, jsl])

    nc.compile()
    return nc


def _get_nc():
    if "nc" not in _CACHE:
        _CACHE["nc"] = _build_program()
    return _CACHE["nc"]


def _make_in_maps(x, W):
    x = np.asarray(x, dtype=np.float32)
    W = np.asarray(W, dtype=np.float32)
    assert x.shape == (B, T, EMB) and W.shape == (EMB, 3 * HEAD)

    wc = W.astype(BF16).reshape(KCH, 128, 192)  # [k, p, c]: [Wk | Wq | Wv]
    wj = (
        np.concatenate([wc, wc[:, :, 0:64]], axis=2)  # append Wk again
        .transpose(1, 0, 2)
        .reshape(128, KCH * 256)
        .copy()
    )
    mask = np.triu(np.ones((128, 128), np.float32)).astype(BF16)
    in_maps = []
    for b in range(B):
        xb = x[b].astype(BF16)  # [T, EMB]
        # xj[j, p, k, c] = x[512j + c, 128k + p]
        xj = np.ascontiguousarray(
            xb.reshape(NTT, 512, KCH, 128).transpose(0, 3, 2, 1)
        )
        in_maps.append({"xj": xj, "w": wj, "mask": mask})
    return in_maps


def _postprocess(o):
    # o: [65, T] fp32 -> [T, HEAD] normalized
    return (o[0:HEAD, :] / o[HEAD:HEAD + 1, :]).T


def kernel(x, W):
    from concourse.bass_utils import run_bass_kernel_spmd

    nc = _get_nc()
    in_maps = _make_in_maps(x, W)
    res = run_bass_kernel_spmd(nc, in_maps, list(range(B)))
    return np.stack(
        [_postprocess(res.results[b]["o"]) for b in range(B)]
    ).astype(np.float32)
